# revision 56
# baseline (speedup 1.0000x reference)
"""Multi-head attention (B=2, N=2048, D=1024, H=16) on 8 TRN2 NeuronCores.

Sharding: tensor-parallel over heads. Core c owns heads 2c, 2c+1 (a 128-wide
slice of the concat head dim). Each core:
  - projects Q^T, K^T (transposed layout [dh, rows]) and V (natural [rows, dh])
    for its heads, over all B*N=4096 rows, from host-transposed bf16 x^T inputs
  - attention with transposed scores S^T[k, q] = K Q^T (row-tiled 64-contraction
    matmul pairs run concurrently on the PE), exp on ScalarE (scale=1/8 folded
    in, no max-subtract needed: |scores/8| < ~4), softmax denominator via an
    ones-block in V (free on TensorE),
  - partial output projection out^T_c = Wo[:, slice] X_c^T  ->  [1024, 4096]
Host sums the 8 partial outputs and adds bo.  bk is dropped on device: a
K-side bias shifts every score of a given query by a constant, which softmax
cancels exactly.

Scheduling: the attention inner loop is software-pipelined so the in-order
TensorE queue never stalls behind ScalarE's exp — scores(kt+1) is issued
before PV(kt), and the out-projection of q-tile i is deferred into q-tile
i+1's loop so the softmax-normalize latency chain (evac/recip/broadcast/mul)
hides completely.  All projection work for the *other* batch is chopped into
~0.5us units and pumped one-per-iteration into the attention loop as TensorE
filler.  Head 1's V values sit in PSUM partitions 64:128 (ones in 0:64,
mirrored from head 0) so both heads' normalize multiplies are lane-aligned
and no cross-partition shift DMA of the values is needed.
"""

import sys

sys.path.insert(0, "/opt/trn_rl_repo")

from contextlib import ExitStack

import ml_dtypes
import numpy as np

import concourse.bass as bass
import concourse.mybir as mybir
import concourse.tile as tile
from concourse import bacc
from concourse.bass_utils import run_bass_kernel_spmd

B, N, D, H, DH = 2, 2048, 1024, 16, 64
R = B * N  # 4096
NC = 8
HPC = H // NC  # 2 heads per core
DHC = HPC * DH  # 128 head dims per core
QT = 512  # query tile (psum bank / fp32 moving max)
KT = 128  # key tile (psum partitions)
NQT = N // QT  # 4
NKT = N // KT  # 16
KC = D // 128  # 8 contraction chunks
XW = 512  # rows per x DMA tile (1KB dma descriptors, finer pipelining)
NCH = N // XW  # 4 chunks per batch

f32 = mybir.dt.float32
bf16 = mybir.dt.bfloat16
fp8 = mybir.dt.float8e4
i8 = mybir.dt.int8

# Schraudolph fast-exp on DVE: bits8 = round(arg * 8*log2e + (56 - 0.45))
# bitcast int8 -> fp8e4m3 approximates exp(arg); arg = score/8 here, so the
# 0.125 scale folds into the multiplier.  Calibrated on hw: ~3.1% RMS.
EXP8_MUL = 0.125 * 1.4426950408889634 * 8.0
EXP8_ADD = 55.55

_cache = {}


def _fold(ap):
    # [D, X] dram -> [128, KC, X] partition-folded view for one-shot DMA
    return ap.rearrange("(a p) m -> p a m", p=128)


def _foldw(w):
    # [D, DHC] host weight -> [128, KC, DHC] partition-folded, contiguous
    return np.ascontiguousarray(w.reshape(KC, 128, DHC).transpose(1, 0, 2))


def build():
    if "nc" in _cache:
        return _cache["nc"]
    nc = bacc.Bacc("TRN2", target_bir_lowering=False, debug=False, num_devices=NC)
    xq = nc.dram_tensor("xqT", [D, R], bf16, kind="ExternalInput").ap()
    xk = nc.dram_tensor("xkT", [D, R], bf16, kind="ExternalInput").ap()
    xv = nc.dram_tensor("xvT", [D, R], bf16, kind="ExternalInput").ap()
    wq = nc.dram_tensor("wqT", [128, KC, DHC], bf16, kind="ExternalInput").ap()
    wk = nc.dram_tensor("wkT", [128, KC, DHC], bf16, kind="ExternalInput").ap()
    wv = nc.dram_tensor("wvT", [128, KC, DHC], bf16, kind="ExternalInput").ap()
    wo = nc.dram_tensor("woT", [DHC, D], bf16, kind="ExternalInput").ap()
    bq = nc.dram_tensor("bq", [DHC, 1], f32, kind="ExternalInput").ap()
    outT = nc.dram_tensor("outT", [D, R], bf16, kind="ExternalOutput").ap()

    with tile.TileContext(nc) as tc, ExitStack() as ctx:
        const = ctx.enter_context(tc.tile_pool(name="const", bufs=1))
        xpool = ctx.enter_context(tc.tile_pool(name="x", bufs=6))
        big = ctx.enter_context(tc.tile_pool(name="big", bufs=1))
        ppool = ctx.enter_context(tc.tile_pool(name="p", bufs=4))
        opool = ctx.enter_context(tc.tile_pool(name="o", bufs=4))
        npool = ctx.enter_context(tc.tile_pool(name="norm", bufs=2))
        ps_proj = ctx.enter_context(tc.tile_pool(name="psA", bufs=2, space="PSUM"))
        ps_s = ctx.enter_context(tc.tile_pool(name="psS", bufs=2, space="PSUM"))
        ps_pv = ctx.enter_context(tc.tile_pool(name="psPV", bufs=2, space="PSUM"))

        # ---- constants: only wk now; the rest are DMA'd lazily as units ----
        wk_sb = const.tile([128, KC, DHC], bf16, tag="wk")
        nc.sync.dma_start(wk_sb[:], wk)
        ones_bc = const.tile([128, 64], bf16, tag="onesbc")
        nc.vector.memset(ones_bc[:], 1.0)
        wv_sb = const.tile([128, KC, DHC], bf16, tag="wv")
        wq_sb = const.tile([128, KC, DHC], bf16, tag="wq")
        wo_sb = const.tile([128, D], bf16, tag="wo")
        bq_sb = const.tile([DHC, 1], f32, tag="bq")

        # ---- per-batch persistent activations ----
        qTs, kTs, vs, xTs = [], [], [], []
        for b in range(B):
            qTs.append(big.tile([128, N], bf16, tag=f"qT{b}", name=f"qT{b}"))
            kTs.append(big.tile([128, N], bf16, tag=f"kT{b}", name=f"kT{b}"))
            # v[:, 0:NKT, :]   head0: values in cols 0:64,  ones in 64:128
            # v[:, NKT:, :]    head1: ones in cols 0:64,  values in 64:128
            # -> head h's PV psum has values in partitions 64h:64h+64 and the
            #    softmax denominator replicated across the other 64 partitions.
            # (bv is dropped on device: softmax weights sum to 1, so the V bias
            #  contributes exactly bv @ Wo.T to the output -- added on host.)
            v = big.tile([128, HPC * NKT, 128], bf16, tag=f"v{b}", name=f"v{b}")
            nc.vector.memset(v[:, 0:NKT, 64:128], 1.0)
            nc.vector.memset(v[:, NKT : 2 * NKT, 0:64], 1.0)
            vs.append(v)
            xTs.append(big.tile([128, N], bf16, tag=f"xT{b}", name=f"xT{b}"))

        # ============ work units: keyed, issued on demand or pumped ============
        units = {}
        order = []
        xtiles = {}

        def register(key, run, cost):
            units[key] = (run, cost)
            order.append(key)

        credit = [0]

        def need(key):
            # issue a unit immediately (no-op if already issued); debit its
            # PE cost from the pump credit so eagerly-pulled work (V proj
            # demanded by pv, K by scores) throttles the voluntary filler
            ent = units.pop(key, None)
            if ent is not None:
                credit[0] -= ent[1]
                ent[0]()

        def pump(budget=550):
            # issue ~one attention-iteration's worth of TensorE slack
            credit[0] = min(credit[0] + budget, 800)
            while order:
                key = order[0]
                if key not in units:
                    order.pop(0)
                    continue
                if units[key][1] > credit[0]:
                    break
                need(order.pop(0))

        def drain():
            while order:
                key = order.pop(0)
                need(key)

        def mk_const_dma(key, sb, dram):
            def run():
                nc.sync.dma_start(sb[:], dram)

            register(key, run, 0)

        def mk_dma_x(xdram, nm, b, xi):
            def run():
                rlo = b * N + xi * XW
                xt = xpool.tile([128, KC, XW], bf16, tag="xt", name="xt")
                src_ap = _fold(xdram[:, rlo : rlo + XW])
                nc.sync.dma_start(xt[:], src_ap)
                xtiles[(nm, b, xi)] = xt

            register((nm, b, xi), run, 0)

        def mk_dma_mini(key, xdram, cols):
            def run():
                xt = const.tile([128, KC, cols], bf16, tag=key[0], name=key[0])
                nc.sync.dma_start(xt[:], _fold(xdram[:, 0:cols]))
                xtiles[key] = xt

            register(key, run, 0)

        mk_dma_mini(("xkm",), xk, QT // 4)
        mk_dma_mini(("xqm",), xq, QT // 2)

        def mk_proj_qk(nm, dst, xnm, b, xi, w_sb, b_sb, wkey):
            # psum[dh2, r] = sum_d W^T[d, dh2] x^T[d, r]  (+ bias in the copy)
            # Emitted as two self-contained 256-col units so a pumped unit
            # never parks more than ~0.9us of matmul in front of the next
            # score pair on the in-order PE queue.
            HW_ = QT // 4

            def run_q(ch):
                def run():
                    need(wkey)
                    if b_sb is not None:
                        need(("cbq",))
                    # the very first K/Q quarters read dedicated mini-DMAs
                    # (0.25/0.5MB) so scores(0) -- and with it the whole
                    # exp-cadence-bound pipeline -- starts ~5us earlier
                    # than the full 1MB chunks allow on the ramping wire
                    if nm == "K" and b == 0 and xi == 0 and ch == 0:
                        need(("xkm",))
                        xt = xtiles[("xkm",)]
                        cs = slice(0, HW_)
                    elif nm == "Q" and b == 0 and xi == 0 and ch < 2:
                        need(("xqm",))
                        xt = xtiles[("xqm",)]
                        cs = slice(ch * HW_, (ch + 1) * HW_)
                    else:
                        need((xnm, b, xi))
                        xt = xtiles[(xnm, b, xi)]
                        cs = slice(ch * HW_, (ch + 1) * HW_)
                    ps = ps_proj.tile([128, HW_], f32, tag="proj", name="psqk")
                    for kc in range(KC):
                        nc.tensor.matmul(
                            ps[:],
                            w_sb[:, kc, :],
                            xt[:, kc, cs],
                            start=(kc == 0),
                            stop=(kc == KC - 1),
                        )
                    d = dst[:, xi * QT + ch * HW_ : xi * QT + (ch + 1) * HW_]
                    if b_sb is None:
                        nc.vector.tensor_copy(d, ps[:])
                    else:
                        nc.vector.tensor_scalar_add(d, ps[:], b_sb[:])

                return run

            for ch in range(4):
                register((nm, b, xi, ch), run_q(ch), 450)
            # alias key for consumers that need the whole 512-col stripe
            def run_all():
                for ch in range(4):
                    need((nm, b, xi, ch))

            register((nm, b, xi), run_all, 0)

        def mk_proj_v(b, xi, rs):
            # natural layout: psum[r, dh2] = sum_d x^T[d, r] W^T[d, dh2]
            def run():
                need(("cwv",))
                need(("xv", b, xi))
                xt = xtiles[("xv", b, xi)]
                ps = ps_proj.tile([128, DHC], f32, tag="proj", name="psv")
                for kc in range(KC):
                    nc.tensor.matmul(
                        ps[:],
                        xt[:, kc, rs * 128 : (rs + 1) * 128],
                        wv_sb[:, kc, :],
                        start=(kc == 0),
                        stop=(kc == KC - 1),
                    )
                kt = xi * (XW // 128) + rs  # key tile index within batch
                nc.vector.tensor_copy(vs[b][:, kt, 0:64], ps[:, 0:64])
                nc.vector.tensor_copy(vs[b][:, NKT + kt, 64:128], ps[:, 64:128])

            register(("V", b, xi * (XW // 128) + rs), run, 500)

        mk_const_dma(("cwv",), wv_sb, wv)
        mk_const_dma(("cwq",), wq_sb, wq)
        mk_const_dma(("cwo",), wo_sb, wo)
        mk_const_dma(("cbq",), bq_sb, bq)

        def register_batch(b):
            # x-chunk DMAs are registered >=2 proj-units ahead of their first
            # consumer so a pumped proj never reaches the PE queue head before
            # its data is resident (PE wait-queue depth is only 4).
            K_ = lambda xi: mk_proj_qk(
                "K", kTs[b], "xk", b, xi, wk_sb, None, ("cwk",)
            )
            Q_ = lambda xi: mk_proj_qk(
                "Q", qTs[b], "xq", b, xi, wq_sb, bq_sb, ("cwq",)
            )
            V_ = lambda v: mk_proj_v(b, v // 4, v % 4)
            mk_dma_x(xk, "xk", b, 0)
            mk_dma_x(xq, "xq", b, 0)
            mk_dma_x(xv, "xv", b, 0)
            K_(0)
            Q_(0)
            mk_dma_x(xk, "xk", b, 1)
            mk_dma_x(xq, "xq", b, 1)
            for v in range(0, 4):
                V_(v)
            K_(1)
            mk_dma_x(xv, "xv", b, 1)
            mk_dma_x(xk, "xk", b, 2)
            Q_(1)
            for v in range(4, 8):
                V_(v)
            mk_dma_x(xq, "xq", b, 2)
            K_(2)
            mk_dma_x(xv, "xv", b, 2)
            Q_(2)
            mk_dma_x(xk, "xk", b, 3)
            for v in range(8, 12):
                V_(v)
            mk_dma_x(xq, "xq", b, 3)
            K_(3)
            mk_dma_x(xv, "xv", b, 3)
            Q_(3)
            for v in range(12, 16):
                V_(v)

        register(("cwk",), lambda: None, 0)  # wk DMA'd at const setup above
        register_batch(0)
        register_batch(1)

        # ===== attention: one global software-pipelined stream =====
        # All q-tiles form a single slot stream; slot i emits scores(i) and
        # pv(i - PVLAG).  The PV of a tile's last key-block is therefore
        # issued AFTER the next tile's first scores, so the PE streams
        # through q-tile boundaries instead of idling on the exp / psum-evac
        # chains.  Normalize + out-projection stages are queued as pendings
        # keyed by global slot.
        pending = []  # (global_slot, fn, pe_cost)

        def outproj_stages(b, qt, op, c0, cw):
            # two out-row blocks (256 contiguous outT rows) as two separate
            # pending stages (one matmul + one evac each) so neither parks
            # >~0.5us of work in front of a score pair; the DMA rides the
            # second stage.
            qs = slice(qt * QT + c0, qt * QT + c0 + cw)
            obx = [None]

            def stage(i):
                def run():
                    need(("cwo",))
                    if i == 0:
                        obx[0] = opool.tile([128, 2, QT], bf16, tag="o", name="ob")
                    ot = op * 2 + i
                    ps = ps_proj.tile([128, QT], f32, tag="proj", name="pso")
                    nc.tensor.matmul(
                        ps[:, 0:cw],
                        wo_sb[:, ot * 128 : (ot + 1) * 128],
                        xTs[b][:, qs],
                        start=True,
                        stop=True,
                    )
                    nc.vector.tensor_copy(obx[0][:, i, 0:cw], ps[:, 0:cw])
                    if i == 1:
                        dst = outT[
                            op * 256 : (op + 1) * 256,
                            b * N + qt * QT + c0 : b * N + qt * QT + c0 + cw,
                        ].rearrange("(a p) m -> p a m", p=128)
                        nc.sync.dma_start(dst, obx[0][:, :, 0:cw])

                return run

            return stage(0), stage(1)

        class Tctx:
            def __init__(self, b, qt, c0, cw):
                self.b, self.qt, self.c0, self.cw = b, qt, c0, cw
                self.qs = slice(qt * QT + c0, qt * QT + c0 + cw)
                self.pvs = None
                self.pts = [None] * NKT

        def mk_scores(ctx, kt):
            # pair matmul: the two heads run concurrently in PE row groups
            # 0/64; one whole-pair exp on ScalarE (the loop's rate limiter).
            b, cw = ctx.b, ctx.cw
            need(("K", b, kt // 4, kt % 4))
            ks = slice(kt * KT, (kt + 1) * KT)
            sg = ps_s.tile([128, 2, QT], f32, tag="sg", name="sg")
            for h in range(HPC):
                hp = slice(64 * h, 64 * h + 64)
                nc.tensor.matmul(
                    sg[:, h, 0:cw],
                    kTs[b][hp, ks],
                    qTs[b][hp, ctx.qs],
                    start=True,
                    stop=True,
                )
            pt = ppool.tile([128, 2, QT], bf16, tag="p", name="pt")
            nc.scalar.activation(
                pt[:, :, 0:cw],
                sg[:, :, 0:cw],
                mybir.ActivationFunctionType.Exp,
                scale=0.125,
            )
            ctx.pts[kt] = pt

        def mk_pv(ctx, kt, g):
            b, cw = ctx.b, ctx.cw
            if kt == 0:
                ctx.pvs = [
                    ps_pv.tile([128, QT], f32, tag="pv", name=f"pv{h}")
                    for h in range(HPC)
                ]
            need(("V", b, kt))
            pt = ctx.pts[kt]
            ctx.pts[kt] = None
            for h in range(HPC):
                nc.tensor.matmul(
                    ctx.pvs[h][:, 0:cw],
                    vs[b][:, h * NKT + kt, :],
                    pt[:, h, 0:cw],
                    start=(kt == 0),
                    stop=(kt == NKT - 1),
                )
            if kt == NKT - 1:
                finalize(ctx, g)

        def finalize(ctx, g):
            # softmax normalize: values for head h live in psum partitions
            # 64h:64h+64; the replicated denominators (from the ones-blocks
            # in V) at partition 64 (h0) / 32 (h1).  Evacuate psum in two
            # copies right away (frees the PV banks for the next tile); the
            # recip/broadcast/mul + out-projection ride the pending queue.
            # bf16 evacuation: values only need bf16 (output is bf16 anyway)
            # and a bf16 denominator costs ~0.1% RMS -- in exchange the rb
            # broadcast matmuls run at bf16 rate (213ns) instead of fp32's
            # LOW_HIGH double-pass (~1.4us), saving ~21us of PE.
            b, cw, qs = ctx.b, ctx.cw, ctx.qs
            pvs = ctx.pvs
            pvsb0 = npool.tile([128, QT], bf16, tag="pvsb0", name="pvsb0")
            nc.vector.tensor_copy(pvsb0[0:65, 0:cw], pvs[0][0:65, 0:cw])
            pvsb1 = npool.tile([128, QT], bf16, tag="pvsb1", name="pvsb1")
            nc.vector.tensor_copy(pvsb1[64:128, 0:cw], pvs[1][64:128, 0:cw])
            nc.vector.tensor_copy(pvsb1[32:33, 0:cw], pvs[1][32:33, 0:cw])
            rbs = [None, None]

            def rb_mms():
                # broadcast the RAW denominator rows across partitions with
                # 1-contraction PE matmuls into PSUM (works from any base
                # partition, unlike gpsimd partition_broadcast / custom-DVE
                # ops, which silently no-op unless the AP starts at 0)
                for h, (pvsb, dp) in enumerate(
                    ((pvsb0, slice(64, 65)), (pvsb1, slice(32, 33)))
                ):
                    rb = ps_proj.tile([128, QT], f32, tag="proj", name=f"rb{h}")
                    nc.tensor.matmul(
                        rb[64 * h : 64 * h + 64, 0:cw],
                        ones_bc[dp, :],
                        pvsb[dp, 0:cw],
                        start=True,
                        stop=True,
                    )
                    rbs[h] = rb

            def recips():
                # full-height base-0 reciprocals (the custom-DVE op silently
                # no-ops unless the AP starts at partition 0)
                nc.vector.reciprocal_approx_fast(
                    rbs[0][0:64, 0:cw], rbs[0][0:64, 0:cw]
                )
                nc.vector.reciprocal_approx_fast(
                    rbs[1][0:128, 0:cw], rbs[1][0:128, 0:cw]
                )

            def muls():
                nc.vector.tensor_mul(
                    xTs[b][0:64, qs], pvsb0[0:64, 0:cw], rbs[0][0:64, 0:cw]
                )
                nc.vector.tensor_mul(
                    xTs[b][64:128, qs], pvsb1[64:128, 0:cw], rbs[1][64:128, 0:cw]
                )

            pending.append((g + 1, rb_mms, 430))
            pending.append((g + 2, recips, 0))
            pending.append((g + 3, muls, 0))
            for op in range(KC // 2):
                s0, s1 = outproj_stages(b, ctx.qt, op, ctx.c0, cw)
                pending.append((g + 4 + 2 * op, s0, 380 * cw // QT))
                pending.append((g + 5 + 2 * op, s1, 380 * cw // QT))

        # ================= schedule =================
        # warm up b0's first tiles in dependency order, then let the stream
        # pull the rest on demand while pump() spreads filler into the loop.
        # DMA issue order = wire priority: the small weight DMAs and the two
        # chunks on the critical path (xk00 -> K00 -> scores, xq00 -> Q00) go
        # first so scores(0) can start ~16us in instead of ~31us.
        need(("cwk",))
        need(("cwq",))
        need(("cbq",))
        need(("xkm",))
        need(("xqm",))
        need(("K", 0, 0, 0))
        need(("Q", 0, 0, 0))
        need(("Q", 0, 0, 1))
        need(("xk", 0, 0))
        need(("K", 0, 0, 1))
        need(("xq", 0, 0))
        need(("xv", 0, 0))
        need(("cwv",))
        need(("K", 0, 0, 2))
        need(("K", 0, 0, 3))
        need(("xk", 0, 1))
        need(("xq", 0, 1))

        # the final q-tile runs as two column halves so the first half's
        # normalize + out-projection overlap the second half's slots; only a
        # half-width serial tail remains after the last matmul
        tiles = [
            (0, 0, 0, QT // 2),
            (0, 0, QT // 2, QT // 2),
            (0, 1, 0, QT),
            (0, 2, 0, QT),
            (0, 3, 0, QT),
            (1, 0, 0, QT),
            (1, 1, 0, QT),
            (1, 2, 0, QT),
            (1, 3, 0, QT // 2),
            (1, 3, QT // 2, QT // 4),
            (1, 3, 3 * QT // 4, QT // 4),
        ]
        # V proj for the first key-tiles so tile (0,0)'s pv's don't pull
        # them eagerly into its already-dense slots
        need(("V", 0, 0))
        need(("V", 0, 1))
        need(("V", 0, 2))

        PVLAG = 2
        slots = []
        ctxs = {}
        tile_start = {}
        for b, qt, c0, cw in tiles:
            ctxs[(b, qt, c0)] = Tctx(b, qt, c0, cw)
            tile_start[(b, qt, c0)] = len(slots)
            for kt in range(NKT):
                slots.append((b, qt, c0, kt))

        # ---- deterministic EDF filler schedule -------------------------
        # Deadline = the slot where a unit is first demanded by the stream.
        # Units are packed into slots by remaining PE-slack capacity in
        # deadline order; an overdue unit is forced regardless of capacity
        # (the stream's need() would pull it there anyway).
        tb = {0: 0, 1: min(s for (bb, qq, cc), s in tile_start.items() if bb == 1)}

        def deadline(key):
            nm = key[0]
            if nm == "V":
                return tb[key[1]] + key[2] + PVLAG - 1
            if nm in ("K", "Q"):
                b, xi = key[1], key[2]
                d = tb[b] + (4 * xi if nm == "K" else 16 * xi) - 1
                if nm == "K" and len(key) == 4:
                    d += key[3]  # quarter ch is first needed at kt = 4*xi+ch
                if nm == "Q" and b == 0 and xi == 0 and len(key) == 4 and key[3] >= 2:
                    d += 16  # second half of the split first tile
                return max(d, 0)
            if nm == "xk":
                return max(tb[key[1]] + 4 * key[2] - 5, 0)
            if nm == "xq":
                return max(tb[key[1]] + 16 * key[2] - 5, 0)
            if nm == "xv":
                return max(tb[key[1]] + 4 * key[2] - 3, 0)
            return 4  # consts (cwo/cwv/...)

        dls = sorted(
            (deadline(k), idx, k)
            for idx, k in enumerate(order)
            if k in units and not (len(k) == 3 and k[0] in ("K", "Q"))
        )
        CAPS = {QT: 360, QT // 2: 260, QT // 4: 180}

        def emit_pv(j, g):
            b, qt, c0, kt = slots[j]
            mk_pv(ctxs[(b, qt, c0)], kt, g)

        ptr = [0]

        def fill(i, cap):
            while ptr[0] < len(dls):
                d, _, key = dls[ptr[0]]
                if key not in units:
                    ptr[0] += 1
                    continue
                cost = units[key][1]
                if d <= i or cost <= cap:
                    cap -= cost
                    need(key)
                    ptr[0] += 1
                else:
                    break

        for i, (b, qt, c0, kt) in enumerate(slots):
            if kt == 0:
                cw_t = ctxs[(b, qt, c0)].cw
                for ch in range(c0 // 128, (c0 + cw_t) // 128):
                    need(("Q", b, qt, ch))
            mk_scores(ctxs[(b, qt, c0)], kt)
            if i >= PVLAG:
                emit_pv(i - PVLAG, i)
            staged_cost = 0
            while pending and pending[0][0] <= i:
                _, fn, cost = pending.pop(0)
                fn()
                staged_cost += cost
            fill(i, CAPS[ctxs[(b, qt, c0)].cw] - staged_cost)
        for j in range(len(slots) - PVLAG, len(slots)):
            emit_pv(j, len(slots))
        drain()
        while pending:
            pending.pop(0)[1]()

    nc.compile()
    _cache["nc"] = nc
    return nc


def kernel(x_q, x_k, x_v, Wq, bq, Wk, bk, Wv, bv, Wo, bo, _trace=False):
    x_q = np.asarray(x_q, dtype=np.float32)
    x_k = np.asarray(x_k, dtype=np.float32)
    x_v = np.asarray(x_v, dtype=np.float32)
    Wq, Wk, Wv, Wo = (np.asarray(w, dtype=np.float32) for w in (Wq, Wk, Wv, Wo))
    bq, bk, bv, bo = (np.asarray(v, dtype=np.float32) for v in (bq, bk, bv, bo))

    bf = ml_dtypes.bfloat16
    xqT = np.ascontiguousarray(x_q.reshape(R, D).T).astype(bf)
    xkT = np.ascontiguousarray(x_k.reshape(R, D).T).astype(bf)
    xvT = np.ascontiguousarray(x_v.reshape(R, D).T).astype(bf)

    in_maps = []
    for c in range(NC):
        s = slice(DHC * c, DHC * (c + 1))
        in_maps.append(
            {
                "xqT": xqT,
                "xkT": xkT,
                "xvT": xvT,
                "wqT": _foldw(Wq[s, :].T).astype(bf),
                "wkT": _foldw(Wk[s, :].T).astype(bf),
                "wvT": _foldw(Wv[s, :].T).astype(bf),
                "woT": np.ascontiguousarray(Wo[:, s].T).astype(bf),
                "bq": bq[s][:, None].copy(),
            }
        )

    nc = build()
    res = run_bass_kernel_spmd(nc, in_maps, core_ids=list(range(NC)), trace=_trace)
    total = np.zeros((D, R), dtype=np.float32)
    for c in range(NC):
        total += res.results[c]["outT"].astype(np.float32)
    # bv is not applied on device: softmax weights sum to 1, so the V bias
    # contributes exactly bv @ Wo.T to every output row -- fold it into bo.
    out = total.T + (bo + bv @ Wo.T)[None, :]
    if _trace:
        kernel.last_exec_time_ns = res.exec_time_ns
    return out.reshape(B, N, D).astype(np.float32)



# revision 57
# speedup vs baseline: 1.0516x; 1.0516x over previous
"""Multi-head attention (B=2, N=2048, D=1024, H=16) on 8 TRN2 NeuronCores.

Sharding: tensor-parallel over heads. Core c owns heads 2c, 2c+1 (a 128-wide
slice of the concat head dim). Each core:
  - projects Q^T, K^T (transposed layout [dh, rows]) and V (natural [rows, dh])
    for its heads, over all B*N=4096 rows, from host-transposed bf16 x^T inputs
  - attention with transposed scores S^T[k, q] = K Q^T (row-tiled 64-contraction
    matmul pairs run concurrently on the PE), exp on ScalarE (scale=1/8 folded
    in, no max-subtract needed: |scores/8| < ~4), softmax denominator via an
    ones-block in V (free on TensorE),
  - partial output projection out^T_c = Wo[:, slice] X_c^T  ->  [1024, 4096]
Host sums the 8 partial outputs and adds bo.  bk is dropped on device: a
K-side bias shifts every score of a given query by a constant, which softmax
cancels exactly.

Scheduling: the attention inner loop is software-pipelined so the in-order
TensorE queue never stalls behind ScalarE's exp — scores(kt+1) is issued
before PV(kt), and the out-projection of q-tile i is deferred into q-tile
i+1's loop so the softmax-normalize latency chain (evac/recip/broadcast/mul)
hides completely.  All projection work for the *other* batch is chopped into
~0.5us units and pumped one-per-iteration into the attention loop as TensorE
filler.  Head 1's V values sit in PSUM partitions 64:128 (ones in 0:64,
mirrored from head 0) so both heads' normalize multiplies are lane-aligned
and no cross-partition shift DMA of the values is needed.
"""

import sys

sys.path.insert(0, "/opt/trn_rl_repo")

from contextlib import ExitStack

import ml_dtypes
import numpy as np

import concourse.bass as bass
import concourse.mybir as mybir
import concourse.tile as tile
from concourse import bacc
from concourse.bass_utils import run_bass_kernel_spmd

B, N, D, H, DH = 2, 2048, 1024, 16, 64
R = B * N  # 4096
NC = 8
HPC = H // NC  # 2 heads per core
DHC = HPC * DH  # 128 head dims per core
QT = 512  # query tile (psum bank / fp32 moving max)
KT = 128  # key tile (psum partitions)
NQT = N // QT  # 4
NKT = N // KT  # 16
KC = D // 128  # 8 contraction chunks
XW = 512  # rows per x DMA tile (1KB dma descriptors, finer pipelining)
NCH = N // XW  # 4 chunks per batch

f32 = mybir.dt.float32
bf16 = mybir.dt.bfloat16
fp8 = mybir.dt.float8e4
i8 = mybir.dt.int8

# Schraudolph fast-exp on DVE: bits8 = round(arg * 8*log2e + (56 - 0.45))
# bitcast int8 -> fp8e4m3 approximates exp(arg); arg = score/8 here, so the
# 0.125 scale folds into the multiplier.  Calibrated on hw: ~3.1% RMS.
EXP8_MUL = 0.125 * 1.4426950408889634 * 8.0
EXP8_ADD = 55.55

_cache = {}


def _fold(ap):
    # [D, X] dram -> [128, KC, X] partition-folded view for one-shot DMA
    return ap.rearrange("(a p) m -> p a m", p=128)


def _foldw(w):
    # [D, DHC] host weight -> [128, KC, DHC] partition-folded, contiguous
    return np.ascontiguousarray(w.reshape(KC, 128, DHC).transpose(1, 0, 2))


def build():
    if "nc" in _cache:
        return _cache["nc"]
    nc = bacc.Bacc("TRN2", target_bir_lowering=False, debug=False, num_devices=NC)
    xq = nc.dram_tensor("xqT", [D, R], bf16, kind="ExternalInput").ap()
    xk = nc.dram_tensor("xkT", [D, R], bf16, kind="ExternalInput").ap()
    xv = nc.dram_tensor("xvT", [D, R], bf16, kind="ExternalInput").ap()
    wq = nc.dram_tensor("wqT", [128, KC, DHC], bf16, kind="ExternalInput").ap()
    wk = nc.dram_tensor("wkT", [128, KC, DHC], bf16, kind="ExternalInput").ap()
    wv = nc.dram_tensor("wvT", [128, KC, DHC], bf16, kind="ExternalInput").ap()
    wo = nc.dram_tensor("woT", [DHC, D], bf16, kind="ExternalInput").ap()
    bq = nc.dram_tensor("bq", [DHC, 1], f32, kind="ExternalInput").ap()
    outT = nc.dram_tensor("outT", [D, R], bf16, kind="ExternalOutput").ap()

    with tile.TileContext(nc) as tc, ExitStack() as ctx:
        const = ctx.enter_context(tc.tile_pool(name="const", bufs=1))
        xpool = ctx.enter_context(tc.tile_pool(name="x", bufs=6))
        big = ctx.enter_context(tc.tile_pool(name="big", bufs=1))
        ppool = ctx.enter_context(tc.tile_pool(name="p", bufs=4))
        opool = ctx.enter_context(tc.tile_pool(name="o", bufs=4))
        npool = ctx.enter_context(tc.tile_pool(name="norm", bufs=2))
        ps_proj = ctx.enter_context(tc.tile_pool(name="psA", bufs=2, space="PSUM"))
        ps_s = ctx.enter_context(tc.tile_pool(name="psS", bufs=2, space="PSUM"))
        ps_pv = ctx.enter_context(tc.tile_pool(name="psPV", bufs=2, space="PSUM"))

        # ---- constants: only wk now; the rest are DMA'd lazily as units ----
        wk_sb = const.tile([128, KC, DHC], bf16, tag="wk")
        nc.sync.dma_start(wk_sb[:], wk)
        ones_bc = const.tile([128, 64], bf16, tag="onesbc")
        nc.vector.memset(ones_bc[:], 1.0)
        wv_sb = const.tile([128, KC, DHC], bf16, tag="wv")
        wq_sb = const.tile([128, KC, DHC], bf16, tag="wq")
        wo_sb = const.tile([128, D], bf16, tag="wo")
        bq_sb = const.tile([DHC, 1], f32, tag="bq")

        # ---- per-batch persistent activations ----
        qTs, kTs, vs, xTs = [], [], [], []
        for b in range(B):
            qTs.append(big.tile([128, N], bf16, tag=f"qT{b}", name=f"qT{b}"))
            kTs.append(big.tile([128, N], bf16, tag=f"kT{b}", name=f"kT{b}"))
            # v[:, 0:NKT, :]   head0: values in cols 0:64,  ones in 64:128
            # v[:, NKT:, :]    head1: ones in cols 0:64,  values in 64:128
            # -> head h's PV psum has values in partitions 64h:64h+64 and the
            #    softmax denominator replicated across the other 64 partitions.
            # (bv is dropped on device: softmax weights sum to 1, so the V bias
            #  contributes exactly bv @ Wo.T to the output -- added on host.)
            v = big.tile([128, HPC * NKT, 128], bf16, tag=f"v{b}", name=f"v{b}")
            nc.vector.memset(v[:, 0:NKT, 64:128], 1.0)
            nc.vector.memset(v[:, NKT : 2 * NKT, 0:64], 1.0)
            vs.append(v)
            xTs.append(big.tile([128, N], bf16, tag=f"xT{b}", name=f"xT{b}"))

        # ============ work units: keyed, issued on demand or pumped ============
        units = {}
        order = []
        xtiles = {}

        def register(key, run, cost):
            units[key] = (run, cost)
            order.append(key)

        credit = [0]

        def need(key):
            # issue a unit immediately (no-op if already issued); debit its
            # PE cost from the pump credit so eagerly-pulled work (V proj
            # demanded by pv, K by scores) throttles the voluntary filler
            ent = units.pop(key, None)
            if ent is not None:
                credit[0] -= ent[1]
                ent[0]()

        def pump(budget=550):
            # issue ~one attention-iteration's worth of TensorE slack
            credit[0] = min(credit[0] + budget, 800)
            while order:
                key = order[0]
                if key not in units:
                    order.pop(0)
                    continue
                if units[key][1] > credit[0]:
                    break
                need(order.pop(0))

        def drain():
            while order:
                key = order.pop(0)
                need(key)

        def mk_const_dma(key, sb, dram):
            def run():
                nc.sync.dma_start(sb[:], dram)

            register(key, run, 0)

        def mk_dma_x(xdram, nm, b, xi):
            def run():
                rlo = b * N + xi * XW
                xt = xpool.tile([128, KC, XW], bf16, tag="xt", name="xt")
                src_ap = _fold(xdram[:, rlo : rlo + XW])
                nc.sync.dma_start(xt[:], src_ap)
                xtiles[(nm, b, xi)] = xt

            register((nm, b, xi), run, 0)

        def mk_dma_mini(key, xdram, cols):
            def run():
                xt = const.tile([128, KC, cols], bf16, tag=key[0], name=key[0])
                nc.sync.dma_start(xt[:], _fold(xdram[:, 0:cols]))
                xtiles[key] = xt

            register(key, run, 0)

        mk_dma_mini(("xkm",), xk, QT // 4)

        def mk_proj_qk(nm, dst, xnm, b, xi, w_sb, b_sb, wkey):
            # psum[dh2, r] = sum_d W^T[d, dh2] x^T[d, r]  (+ bias in the copy)
            # Emitted as two self-contained 256-col units so a pumped unit
            # never parks more than ~0.9us of matmul in front of the next
            # score pair on the in-order PE queue.
            HW_ = QT // 4

            def run_q(ch):
                def run():
                    need(wkey)
                    if b_sb is not None:
                        need(("cbq",))
                    # the very first K/Q quarters read dedicated mini-DMAs
                    # (0.25/0.5MB) so scores(0) -- and with it the whole
                    # exp-cadence-bound pipeline -- starts ~5us earlier
                    # than the full 1MB chunks allow on the ramping wire
                    if nm == "K" and b == 0 and xi == 0 and ch == 0:
                        need(("xkm",))
                        xt = xtiles[("xkm",)]
                        cs = slice(0, HW_)
                    else:
                        need((xnm, b, xi))
                        xt = xtiles[(xnm, b, xi)]
                        cs = slice(ch * HW_, (ch + 1) * HW_)
                    ps = ps_proj.tile([128, HW_], f32, tag="proj", name="psqk")
                    for kc in range(KC):
                        nc.tensor.matmul(
                            ps[:],
                            w_sb[:, kc, :],
                            xt[:, kc, cs],
                            start=(kc == 0),
                            stop=(kc == KC - 1),
                        )
                    d = dst[:, xi * QT + ch * HW_ : xi * QT + (ch + 1) * HW_]
                    if b_sb is None:
                        nc.vector.tensor_copy(d, ps[:])
                    else:
                        nc.vector.tensor_scalar_add(d, ps[:], b_sb[:])

                return run

            for ch in range(4):
                register((nm, b, xi, ch), run_q(ch), 450)
            # alias key for consumers that need the whole 512-col stripe
            def run_all():
                for ch in range(4):
                    need((nm, b, xi, ch))

            register((nm, b, xi), run_all, 0)

        def mk_proj_v(b, xi, rs):
            # natural layout: psum[r, dh2] = sum_d x^T[d, r] W^T[d, dh2]
            def run():
                need(("cwv",))
                need(("xv", b, xi))
                xt = xtiles[("xv", b, xi)]
                ps = ps_proj.tile([128, DHC], f32, tag="proj", name="psv")
                for kc in range(KC):
                    nc.tensor.matmul(
                        ps[:],
                        xt[:, kc, rs * 128 : (rs + 1) * 128],
                        wv_sb[:, kc, :],
                        start=(kc == 0),
                        stop=(kc == KC - 1),
                    )
                kt = xi * (XW // 128) + rs  # key tile index within batch
                nc.vector.tensor_copy(vs[b][:, kt, 0:64], ps[:, 0:64])
                nc.vector.tensor_copy(vs[b][:, NKT + kt, 64:128], ps[:, 64:128])

            register(("V", b, xi * (XW // 128) + rs), run, 500)

        mk_const_dma(("cwv",), wv_sb, wv)
        mk_const_dma(("cwq",), wq_sb, wq)
        mk_const_dma(("cwo",), wo_sb, wo)
        mk_const_dma(("cbq",), bq_sb, bq)

        def register_batch(b):
            # x-chunk DMAs are registered >=2 proj-units ahead of their first
            # consumer so a pumped proj never reaches the PE queue head before
            # its data is resident (PE wait-queue depth is only 4).
            K_ = lambda xi: mk_proj_qk(
                "K", kTs[b], "xk", b, xi, wk_sb, None, ("cwk",)
            )
            Q_ = lambda xi: mk_proj_qk(
                "Q", qTs[b], "xq", b, xi, wq_sb, bq_sb, ("cwq",)
            )
            V_ = lambda v: mk_proj_v(b, v // 4, v % 4)
            mk_dma_x(xk, "xk", b, 0)
            mk_dma_x(xq, "xq", b, 0)
            mk_dma_x(xv, "xv", b, 0)
            K_(0)
            Q_(0)
            mk_dma_x(xk, "xk", b, 1)
            mk_dma_x(xq, "xq", b, 1)
            for v in range(0, 4):
                V_(v)
            K_(1)
            mk_dma_x(xv, "xv", b, 1)
            mk_dma_x(xk, "xk", b, 2)
            Q_(1)
            for v in range(4, 8):
                V_(v)
            mk_dma_x(xq, "xq", b, 2)
            K_(2)
            mk_dma_x(xv, "xv", b, 2)
            Q_(2)
            mk_dma_x(xk, "xk", b, 3)
            for v in range(8, 12):
                V_(v)
            mk_dma_x(xq, "xq", b, 3)
            K_(3)
            mk_dma_x(xv, "xv", b, 3)
            Q_(3)
            for v in range(12, 16):
                V_(v)

        register(("cwk",), lambda: None, 0)  # wk DMA'd at const setup above
        register_batch(0)
        register_batch(1)

        # ===== attention: one global software-pipelined stream =====
        # All q-tiles form a single slot stream; slot i emits scores(i) and
        # pv(i - PVLAG).  The PV of a tile's last key-block is therefore
        # issued AFTER the next tile's first scores, so the PE streams
        # through q-tile boundaries instead of idling on the exp / psum-evac
        # chains.  Normalize + out-projection stages are queued as pendings
        # keyed by global slot.
        pending = []  # (global_slot, fn, pe_cost)

        def outproj_stages(b, qt, op, c0, cw):
            # two out-row blocks (256 contiguous outT rows) as two separate
            # pending stages (one matmul + one evac each) so neither parks
            # >~0.5us of work in front of a score pair; the DMA rides the
            # second stage.
            qs = slice(qt * QT + c0, qt * QT + c0 + cw)
            obx = [None]

            def stage(i):
                def run():
                    need(("cwo",))
                    if i == 0:
                        obx[0] = opool.tile([128, 2, QT], bf16, tag="o", name="ob")
                    ot = op * 2 + i
                    ps = ps_proj.tile([128, QT], f32, tag="proj", name="pso")
                    nc.tensor.matmul(
                        ps[:, 0:cw],
                        wo_sb[:, ot * 128 : (ot + 1) * 128],
                        xTs[b][:, qs],
                        start=True,
                        stop=True,
                    )
                    nc.vector.tensor_copy(obx[0][:, i, 0:cw], ps[:, 0:cw])
                    if i == 1:
                        dst = outT[
                            op * 256 : (op + 1) * 256,
                            b * N + qt * QT + c0 : b * N + qt * QT + c0 + cw,
                        ].rearrange("(a p) m -> p a m", p=128)
                        nc.sync.dma_start(dst, obx[0][:, :, 0:cw])

                return run

            return stage(0), stage(1)

        class Tctx:
            def __init__(self, b, qt, c0, cw):
                self.b, self.qt, self.c0, self.cw = b, qt, c0, cw
                self.qs = slice(qt * QT + c0, qt * QT + c0 + cw)
                self.pvs = None
                self.pts = [None] * NKT

        def mk_scores(ctx, kt):
            # pair matmul: the two heads run concurrently in PE row groups
            # 0/64; one whole-pair exp on ScalarE (the loop's rate limiter).
            b, cw = ctx.b, ctx.cw
            need(("K", b, kt // 4, kt % 4))
            ks = slice(kt * KT, (kt + 1) * KT)
            sg = ps_s.tile([128, 2, QT], f32, tag="sg", name="sg")
            for h in range(HPC):
                hp = slice(64 * h, 64 * h + 64)
                nc.tensor.matmul(
                    sg[:, h, 0:cw],
                    kTs[b][hp, ks],
                    qTs[b][hp, ctx.qs],
                    start=True,
                    stop=True,
                )
            pt = ppool.tile([128, 2, QT], bf16, tag="p", name="pt")
            nc.scalar.activation(
                pt[:, :, 0:cw],
                sg[:, :, 0:cw],
                mybir.ActivationFunctionType.Exp,
                scale=0.125,
            )
            ctx.pts[kt] = pt

        def mk_pv(ctx, kt, g):
            b, cw = ctx.b, ctx.cw
            if kt == 0:
                ctx.pvs = [
                    ps_pv.tile([128, QT], f32, tag="pv", name=f"pv{h}")
                    for h in range(HPC)
                ]
            need(("V", b, kt))
            pt = ctx.pts[kt]
            ctx.pts[kt] = None
            for h in range(HPC):
                nc.tensor.matmul(
                    ctx.pvs[h][:, 0:cw],
                    vs[b][:, h * NKT + kt, :],
                    pt[:, h, 0:cw],
                    start=(kt == 0),
                    stop=(kt == NKT - 1),
                )
            if kt == NKT - 1:
                finalize(ctx, g)

        def finalize(ctx, g):
            # softmax normalize: values for head h live in psum partitions
            # 64h:64h+64; the replicated denominators (from the ones-blocks
            # in V) at partition 64 (h0) / 32 (h1).  Evacuate psum in two
            # copies right away (frees the PV banks for the next tile); the
            # recip/broadcast/mul + out-projection ride the pending queue.
            # bf16 evacuation: values only need bf16 (output is bf16 anyway)
            # and a bf16 denominator costs ~0.1% RMS -- in exchange the rb
            # broadcast matmuls run at bf16 rate (213ns) instead of fp32's
            # LOW_HIGH double-pass (~1.4us), saving ~21us of PE.
            b, cw, qs = ctx.b, ctx.cw, ctx.qs
            pvs = ctx.pvs
            pvsb0 = npool.tile([128, QT], bf16, tag="pvsb0", name="pvsb0")
            nc.vector.tensor_copy(pvsb0[0:65, 0:cw], pvs[0][0:65, 0:cw])
            pvsb1 = npool.tile([128, QT], bf16, tag="pvsb1", name="pvsb1")
            nc.vector.tensor_copy(pvsb1[64:128, 0:cw], pvs[1][64:128, 0:cw])
            nc.vector.tensor_copy(pvsb1[32:33, 0:cw], pvs[1][32:33, 0:cw])
            rbs = [None, None]

            def rb_mms():
                # broadcast the RAW denominator rows across partitions with
                # 1-contraction PE matmuls into PSUM (works from any base
                # partition, unlike gpsimd partition_broadcast / custom-DVE
                # ops, which silently no-op unless the AP starts at 0)
                for h, (pvsb, dp) in enumerate(
                    ((pvsb0, slice(64, 65)), (pvsb1, slice(32, 33)))
                ):
                    rb = ps_proj.tile([128, QT], f32, tag="proj", name=f"rb{h}")
                    nc.tensor.matmul(
                        rb[64 * h : 64 * h + 64, 0:cw],
                        ones_bc[dp, :],
                        pvsb[dp, 0:cw],
                        start=True,
                        stop=True,
                    )
                    rbs[h] = rb

            def recips():
                # full-height base-0 reciprocals (the custom-DVE op silently
                # no-ops unless the AP starts at partition 0)
                nc.vector.reciprocal_approx_fast(
                    rbs[0][0:64, 0:cw], rbs[0][0:64, 0:cw]
                )
                nc.vector.reciprocal_approx_fast(
                    rbs[1][0:128, 0:cw], rbs[1][0:128, 0:cw]
                )

            def muls():
                nc.vector.tensor_mul(
                    xTs[b][0:64, qs], pvsb0[0:64, 0:cw], rbs[0][0:64, 0:cw]
                )
                nc.vector.tensor_mul(
                    xTs[b][64:128, qs], pvsb1[64:128, 0:cw], rbs[1][64:128, 0:cw]
                )

            pending.append((g + 1, rb_mms, 430))
            pending.append((g + 2, recips, 0))
            pending.append((g + 3, muls, 0))
            for op in range(KC // 2):
                s0, s1 = outproj_stages(b, ctx.qt, op, ctx.c0, cw)
                pending.append((g + 4 + 2 * op, s0, 380 * cw // QT))
                pending.append((g + 5 + 2 * op, s1, 380 * cw // QT))

        # ================= schedule =================
        # warm up b0's first tiles in dependency order, then let the stream
        # pull the rest on demand while pump() spreads filler into the loop.
        # DMA issue order = wire priority: the small weight DMAs and the two
        # chunks on the critical path (xk00 -> K00 -> scores, xq00 -> Q00) go
        # first so scores(0) can start ~16us in instead of ~31us.
        need(("cwk",))
        need(("cwq",))
        need(("cbq",))
        need(("xkm",))
        need(("xq", 0, 0))
        need(("K", 0, 0, 0))
        need(("Q", 0, 0))
        need(("xk", 0, 0))
        need(("K", 0, 0, 1))
        need(("xv", 0, 0))
        need(("cwv",))
        need(("K", 0, 0, 2))
        need(("K", 0, 0, 3))
        need(("xk", 0, 1))
        need(("xq", 0, 1))

        # the final q-tile runs as two column halves so the first half's
        # normalize + out-projection overlap the second half's slots; only a
        # half-width serial tail remains after the last matmul
        tiles = [
            (0, 0, 0, QT),
            (0, 1, 0, QT),
            (0, 2, 0, QT),
            (0, 3, 0, QT),
            (1, 0, 0, QT),
            (1, 1, 0, QT),
            (1, 2, 0, QT),
            (1, 3, 0, QT // 2),
            (1, 3, QT // 2, QT // 4),
            (1, 3, 3 * QT // 4, QT // 4),
        ]
        # V proj for the first key-tiles so tile (0,0)'s pv's don't pull
        # them eagerly into its already-dense slots
        need(("V", 0, 0))
        need(("V", 0, 1))
        need(("V", 0, 2))

        PVLAG = 2
        slots = []
        ctxs = {}
        tile_start = {}
        for b, qt, c0, cw in tiles:
            ctxs[(b, qt, c0)] = Tctx(b, qt, c0, cw)
            tile_start[(b, qt, c0)] = len(slots)
            for kt in range(NKT):
                slots.append((b, qt, c0, kt))

        # ---- deterministic EDF filler schedule -------------------------
        # Deadline = the slot where a unit is first demanded by the stream.
        # Units are packed into slots by remaining PE-slack capacity in
        # deadline order; an overdue unit is forced regardless of capacity
        # (the stream's need() would pull it there anyway).
        tb = {0: 0, 1: min(s for (bb, qq, cc), s in tile_start.items() if bb == 1)}

        def deadline(key):
            nm = key[0]
            if nm == "V":
                return tb[key[1]] + key[2] + PVLAG - 1
            if nm in ("K", "Q"):
                b, xi = key[1], key[2]
                d = tb[b] + (4 * xi if nm == "K" else 16 * xi) - 1
                if nm == "K" and len(key) == 4:
                    d += key[3]  # quarter ch is first needed at kt = 4*xi+ch
                return max(d, 0)
            if nm == "xk":
                return max(tb[key[1]] + 4 * key[2] - 5, 0)
            if nm == "xq":
                return max(tb[key[1]] + 16 * key[2] - 5, 0)
            if nm == "xv":
                return max(tb[key[1]] + 4 * key[2] - 3, 0)
            return 4  # consts (cwo/cwv/...)

        dls = sorted(
            (deadline(k), idx, k)
            for idx, k in enumerate(order)
            if k in units and not (len(k) == 3 and k[0] in ("K", "Q"))
        )
        CAPS = {QT: 360, QT // 2: 260, QT // 4: 180}

        def emit_pv(j, g):
            b, qt, c0, kt = slots[j]
            mk_pv(ctxs[(b, qt, c0)], kt, g)

        ptr = [0]

        def fill(i, cap):
            while ptr[0] < len(dls):
                d, _, key = dls[ptr[0]]
                if key not in units:
                    ptr[0] += 1
                    continue
                cost = units[key][1]
                if d <= i or cost <= cap:
                    cap -= cost
                    need(key)
                    ptr[0] += 1
                else:
                    break

        for i, (b, qt, c0, kt) in enumerate(slots):
            if kt == 0:
                cw_t = ctxs[(b, qt, c0)].cw
                for ch in range(c0 // 128, (c0 + cw_t) // 128):
                    need(("Q", b, qt, ch))
            mk_scores(ctxs[(b, qt, c0)], kt)
            if i >= PVLAG:
                emit_pv(i - PVLAG, i)
            staged_cost = 0
            while pending and pending[0][0] <= i:
                _, fn, cost = pending.pop(0)
                fn()
                staged_cost += cost
            fill(i, CAPS[ctxs[(b, qt, c0)].cw] - staged_cost)
        for j in range(len(slots) - PVLAG, len(slots)):
            emit_pv(j, len(slots))
        drain()
        while pending:
            pending.pop(0)[1]()

    nc.compile()
    _cache["nc"] = nc
    return nc


def kernel(x_q, x_k, x_v, Wq, bq, Wk, bk, Wv, bv, Wo, bo, _trace=False):
    x_q = np.asarray(x_q, dtype=np.float32)
    x_k = np.asarray(x_k, dtype=np.float32)
    x_v = np.asarray(x_v, dtype=np.float32)
    Wq, Wk, Wv, Wo = (np.asarray(w, dtype=np.float32) for w in (Wq, Wk, Wv, Wo))
    bq, bk, bv, bo = (np.asarray(v, dtype=np.float32) for v in (bq, bk, bv, bo))

    bf = ml_dtypes.bfloat16
    xqT = np.ascontiguousarray(x_q.reshape(R, D).T).astype(bf)
    xkT = np.ascontiguousarray(x_k.reshape(R, D).T).astype(bf)
    xvT = np.ascontiguousarray(x_v.reshape(R, D).T).astype(bf)

    in_maps = []
    for c in range(NC):
        s = slice(DHC * c, DHC * (c + 1))
        in_maps.append(
            {
                "xqT": xqT,
                "xkT": xkT,
                "xvT": xvT,
                "wqT": _foldw(Wq[s, :].T).astype(bf),
                "wkT": _foldw(Wk[s, :].T).astype(bf),
                "wvT": _foldw(Wv[s, :].T).astype(bf),
                "woT": np.ascontiguousarray(Wo[:, s].T).astype(bf),
                "bq": bq[s][:, None].copy(),
            }
        )

    nc = build()
    res = run_bass_kernel_spmd(nc, in_maps, core_ids=list(range(NC)), trace=_trace)
    total = np.zeros((D, R), dtype=np.float32)
    for c in range(NC):
        total += res.results[c]["outT"].astype(np.float32)
    # bv is not applied on device: softmax weights sum to 1, so the V bias
    # contributes exactly bv @ Wo.T to every output row -- fold it into bo.
    out = total.T + (bo + bv @ Wo.T)[None, :]
    if _trace:
        kernel.last_exec_time_ns = res.exec_time_ns
    return out.reshape(B, N, D).astype(np.float32)



# revision 58
# speedup vs baseline: 1.0688x; 1.0163x over previous
"""Multi-head attention (B=2, N=2048, D=1024, H=16) on 8 TRN2 NeuronCores.

Sharding: tensor-parallel over heads. Core c owns heads 2c, 2c+1 (a 128-wide
slice of the concat head dim). Each core:
  - projects Q^T, K^T (transposed layout [dh, rows]) and V (natural [rows, dh])
    for its heads, over all B*N=4096 rows, from host-transposed bf16 x^T inputs
  - attention with transposed scores S^T[k, q] = K Q^T (row-tiled 64-contraction
    matmul pairs run concurrently on the PE), exp on ScalarE (scale=1/8 folded
    in, no max-subtract needed: |scores/8| < ~4), softmax denominator via an
    ones-block in V (free on TensorE),
  - partial output projection out^T_c = Wo[:, slice] X_c^T  ->  [1024, 4096]
Host sums the 8 partial outputs and adds bo.  bk is dropped on device: a
K-side bias shifts every score of a given query by a constant, which softmax
cancels exactly.

Scheduling: the attention inner loop is software-pipelined so the in-order
TensorE queue never stalls behind ScalarE's exp — scores(kt+1) is issued
before PV(kt), and the out-projection of q-tile i is deferred into q-tile
i+1's loop so the softmax-normalize latency chain (evac/recip/broadcast/mul)
hides completely.  All projection work for the *other* batch is chopped into
~0.5us units and pumped one-per-iteration into the attention loop as TensorE
filler.  Head 1's V values sit in PSUM partitions 64:128 (ones in 0:64,
mirrored from head 0) so both heads' normalize multiplies are lane-aligned
and no cross-partition shift DMA of the values is needed.
"""

import sys

sys.path.insert(0, "/opt/trn_rl_repo")

from contextlib import ExitStack

import ml_dtypes
import numpy as np

import concourse.bass as bass
import concourse.mybir as mybir
import concourse.tile as tile
from concourse import bacc
from concourse.bass_utils import run_bass_kernel_spmd

B, N, D, H, DH = 2, 2048, 1024, 16, 64
R = B * N  # 4096
NC = 8
HPC = H // NC  # 2 heads per core
DHC = HPC * DH  # 128 head dims per core
QT = 512  # query tile (psum bank / fp32 moving max)
KT = 128  # key tile (psum partitions)
NQT = N // QT  # 4
NKT = N // KT  # 16
KC = D // 128  # 8 contraction chunks
XW = 512  # rows per x DMA tile (1KB dma descriptors, finer pipelining)
NCH = N // XW  # 4 chunks per batch

f32 = mybir.dt.float32
bf16 = mybir.dt.bfloat16
fp8 = mybir.dt.float8e4
i8 = mybir.dt.int8

# Schraudolph fast-exp on DVE: bits8 = round(arg * 8*log2e + (56 - 0.45))
# bitcast int8 -> fp8e4m3 approximates exp(arg); arg = score/8 here, so the
# 0.125 scale folds into the multiplier.  Calibrated on hw: ~3.1% RMS.
EXP8_MUL = 0.125 * 1.4426950408889634 * 8.0
EXP8_ADD = 55.55

_cache = {}


def _fold(ap):
    # [D, X] dram -> [128, KC, X] partition-folded view for one-shot DMA
    return ap.rearrange("(a p) m -> p a m", p=128)


def _foldw(w):
    # [D, DHC] host weight -> [128, KC, DHC] partition-folded, contiguous
    return np.ascontiguousarray(w.reshape(KC, 128, DHC).transpose(1, 0, 2))


def build():
    if "nc" in _cache:
        return _cache["nc"]
    nc = bacc.Bacc("TRN2", target_bir_lowering=False, debug=False, num_devices=NC)
    xq = nc.dram_tensor("xqT", [D, R], bf16, kind="ExternalInput").ap()
    xk = nc.dram_tensor("xkT", [D, R], bf16, kind="ExternalInput").ap()
    xv = nc.dram_tensor("xvT", [D, R], bf16, kind="ExternalInput").ap()
    wq = nc.dram_tensor("wqT", [128, KC, DHC], bf16, kind="ExternalInput").ap()
    wk = nc.dram_tensor("wkT", [128, KC, DHC], bf16, kind="ExternalInput").ap()
    wv = nc.dram_tensor("wvT", [128, KC, DHC], bf16, kind="ExternalInput").ap()
    wo = nc.dram_tensor("woT", [DHC, D], bf16, kind="ExternalInput").ap()
    bq = nc.dram_tensor("bq", [DHC, 1], f32, kind="ExternalInput").ap()
    outT = nc.dram_tensor("outT", [D, R], bf16, kind="ExternalOutput").ap()

    with tile.TileContext(nc) as tc, ExitStack() as ctx:
        const = ctx.enter_context(tc.tile_pool(name="const", bufs=1))
        xpool = ctx.enter_context(tc.tile_pool(name="x", bufs=6))
        big = ctx.enter_context(tc.tile_pool(name="big", bufs=1))
        ppool = ctx.enter_context(tc.tile_pool(name="p", bufs=4))
        opool = ctx.enter_context(tc.tile_pool(name="o", bufs=4))
        npool = ctx.enter_context(tc.tile_pool(name="norm", bufs=2))
        ps_proj = ctx.enter_context(tc.tile_pool(name="psA", bufs=2, space="PSUM"))
        ps_s = ctx.enter_context(tc.tile_pool(name="psS", bufs=2, space="PSUM"))
        ps_pv = ctx.enter_context(tc.tile_pool(name="psPV", bufs=2, space="PSUM"))

        # ---- constants: only wk now; the rest are DMA'd lazily as units ----
        wk_sb = const.tile([128, KC, DHC], bf16, tag="wk")
        nc.sync.dma_start(wk_sb[:], wk)
        ones_bc = const.tile([128, 64], bf16, tag="onesbc")
        nc.vector.memset(ones_bc[:], 1.0)
        wv_sb = const.tile([128, KC, DHC], bf16, tag="wv")
        wq_sb = const.tile([128, KC, DHC], bf16, tag="wq")
        wo_sb = const.tile([128, D], bf16, tag="wo")
        bq_sb = const.tile([DHC, 1], f32, tag="bq")

        # ---- per-batch persistent activations ----
        qTs, kTs, vs, xTs = [], [], [], []
        for b in range(B):
            qTs.append(big.tile([128, N], bf16, tag=f"qT{b}", name=f"qT{b}"))
            kTs.append(big.tile([128, N], bf16, tag=f"kT{b}", name=f"kT{b}"))
            # v[:, 0:NKT, :]   head0: values in cols 0:64,  ones in 64:128
            # v[:, NKT:, :]    head1: ones in cols 0:64,  values in 64:128
            # -> head h's PV psum has values in partitions 64h:64h+64 and the
            #    softmax denominator replicated across the other 64 partitions.
            # (bv is dropped on device: softmax weights sum to 1, so the V bias
            #  contributes exactly bv @ Wo.T to the output -- added on host.)
            v = big.tile([128, HPC * NKT, 128], bf16, tag=f"v{b}", name=f"v{b}")
            nc.vector.memset(v[:, 0:NKT, 64:128], 1.0)
            nc.vector.memset(v[:, NKT : 2 * NKT, 0:64], 1.0)
            vs.append(v)
            xTs.append(big.tile([128, N], bf16, tag=f"xT{b}", name=f"xT{b}"))

        # ============ work units: keyed, issued on demand or pumped ============
        units = {}
        order = []
        xtiles = {}

        def register(key, run, cost):
            units[key] = (run, cost)
            order.append(key)

        credit = [0]

        def need(key):
            # issue a unit immediately (no-op if already issued); debit its
            # PE cost from the pump credit so eagerly-pulled work (V proj
            # demanded by pv, K by scores) throttles the voluntary filler
            ent = units.pop(key, None)
            if ent is not None:
                credit[0] -= ent[1]
                ent[0]()

        def pump(budget=550):
            # issue ~one attention-iteration's worth of TensorE slack
            credit[0] = min(credit[0] + budget, 800)
            while order:
                key = order[0]
                if key not in units:
                    order.pop(0)
                    continue
                if units[key][1] > credit[0]:
                    break
                need(order.pop(0))

        def drain():
            while order:
                key = order.pop(0)
                need(key)

        def mk_const_dma(key, sb, dram):
            def run():
                nc.sync.dma_start(sb[:], dram)

            register(key, run, 0)

        def mk_dma_x(xdram, nm, b, xi):
            def run():
                rlo = b * N + xi * XW
                xt = xpool.tile([128, KC, XW], bf16, tag="xt", name="xt")
                src_ap = _fold(xdram[:, rlo : rlo + XW])
                nc.sync.dma_start(xt[:], src_ap)
                xtiles[(nm, b, xi)] = xt

            register((nm, b, xi), run, 0)

        def mk_dma_mini(key, xdram, cols):
            def run():
                xt = const.tile([128, KC, cols], bf16, tag=key[0], name=key[0])
                nc.sync.dma_start(xt[:], _fold(xdram[:, 0:cols]))
                xtiles[key] = xt

            register(key, run, 0)

        mk_dma_mini(("xkm",), xk, QT // 4)

        def mk_proj_qk(nm, dst, xnm, b, xi, w_sb, b_sb, wkey):
            # psum[dh2, r] = sum_d W^T[d, dh2] x^T[d, r]  (+ bias in the copy)
            # Emitted as two self-contained 256-col units so a pumped unit
            # never parks more than ~0.9us of matmul in front of the next
            # score pair on the in-order PE queue.
            HW_ = QT // 4

            def run_q(ch):
                def run():
                    need(wkey)
                    if b_sb is not None:
                        need(("cbq",))
                    # the very first K/Q quarters read dedicated mini-DMAs
                    # (0.25/0.5MB) so scores(0) -- and with it the whole
                    # exp-cadence-bound pipeline -- starts ~5us earlier
                    # than the full 1MB chunks allow on the ramping wire
                    if nm == "K" and b == 0 and xi == 0 and ch == 0:
                        need(("xkm",))
                        xt = xtiles[("xkm",)]
                        cs = slice(0, HW_)
                    else:
                        need((xnm, b, xi))
                        xt = xtiles[(xnm, b, xi)]
                        cs = slice(ch * HW_, (ch + 1) * HW_)
                    ps = ps_proj.tile([128, HW_], f32, tag="proj", name="psqk")
                    for kc in range(KC):
                        nc.tensor.matmul(
                            ps[:],
                            w_sb[:, kc, :],
                            xt[:, kc, cs],
                            start=(kc == 0),
                            stop=(kc == KC - 1),
                        )
                    d = dst[:, xi * QT + ch * HW_ : xi * QT + (ch + 1) * HW_]
                    if b_sb is None:
                        nc.vector.tensor_copy(d, ps[:])
                    else:
                        nc.vector.tensor_scalar_add(d, ps[:], b_sb[:])

                return run

            for ch in range(4):
                register((nm, b, xi, ch), run_q(ch), 450)
            # alias key for consumers that need the whole 512-col stripe
            def run_all():
                for ch in range(4):
                    need((nm, b, xi, ch))

            register((nm, b, xi), run_all, 0)

        def mk_proj_v(b, xi, rs):
            # natural layout: psum[r, dh2] = sum_d x^T[d, r] W^T[d, dh2]
            def run():
                need(("cwv",))
                need(("xv", b, xi))
                xt = xtiles[("xv", b, xi)]
                ps = ps_proj.tile([128, DHC], f32, tag="proj", name="psv")
                for kc in range(KC):
                    nc.tensor.matmul(
                        ps[:],
                        xt[:, kc, rs * 128 : (rs + 1) * 128],
                        wv_sb[:, kc, :],
                        start=(kc == 0),
                        stop=(kc == KC - 1),
                    )
                kt = xi * (XW // 128) + rs  # key tile index within batch
                nc.vector.tensor_copy(vs[b][:, kt, 0:64], ps[:, 0:64])
                nc.vector.tensor_copy(vs[b][:, NKT + kt, 64:128], ps[:, 64:128])

            register(("V", b, xi * (XW // 128) + rs), run, 500)

        mk_const_dma(("cwv",), wv_sb, wv)
        mk_const_dma(("cwq",), wq_sb, wq)
        mk_const_dma(("cwo",), wo_sb, wo)
        mk_const_dma(("cbq",), bq_sb, bq)

        def register_batch(b):
            # x-chunk DMAs are registered >=2 proj-units ahead of their first
            # consumer so a pumped proj never reaches the PE queue head before
            # its data is resident (PE wait-queue depth is only 4).
            K_ = lambda xi: mk_proj_qk(
                "K", kTs[b], "xk", b, xi, wk_sb, None, ("cwk",)
            )
            Q_ = lambda xi: mk_proj_qk(
                "Q", qTs[b], "xq", b, xi, wq_sb, bq_sb, ("cwq",)
            )
            V_ = lambda v: mk_proj_v(b, v // 4, v % 4)
            mk_dma_x(xk, "xk", b, 0)
            mk_dma_x(xq, "xq", b, 0)
            mk_dma_x(xv, "xv", b, 0)
            K_(0)
            Q_(0)
            mk_dma_x(xk, "xk", b, 1)
            mk_dma_x(xq, "xq", b, 1)
            for v in range(0, 4):
                V_(v)
            K_(1)
            mk_dma_x(xv, "xv", b, 1)
            mk_dma_x(xk, "xk", b, 2)
            Q_(1)
            for v in range(4, 8):
                V_(v)
            mk_dma_x(xq, "xq", b, 2)
            K_(2)
            mk_dma_x(xv, "xv", b, 2)
            Q_(2)
            mk_dma_x(xk, "xk", b, 3)
            for v in range(8, 12):
                V_(v)
            mk_dma_x(xq, "xq", b, 3)
            K_(3)
            mk_dma_x(xv, "xv", b, 3)
            Q_(3)
            for v in range(12, 16):
                V_(v)

        register(("cwk",), lambda: None, 0)  # wk DMA'd at const setup above
        register_batch(0)
        register_batch(1)

        # ===== attention: one global software-pipelined stream =====
        # All q-tiles form a single slot stream; slot i emits scores(i) and
        # pv(i - PVLAG).  The PV of a tile's last key-block is therefore
        # issued AFTER the next tile's first scores, so the PE streams
        # through q-tile boundaries instead of idling on the exp / psum-evac
        # chains.  Normalize + out-projection stages are queued as pendings
        # keyed by global slot.
        pending = []  # (global_slot, fn, pe_cost)

        def outproj_stages(b, qt, op, c0, cw):
            # two out-row blocks (256 contiguous outT rows) as two separate
            # pending stages (one matmul + one evac each) so neither parks
            # >~0.5us of work in front of a score pair; the DMA rides the
            # second stage.
            qs = slice(qt * QT + c0, qt * QT + c0 + cw)
            obx = [None]

            def stage(i):
                def run():
                    need(("cwo",))
                    if i == 0:
                        obx[0] = opool.tile([128, 2, QT], bf16, tag="o", name="ob")
                    ot = op * 2 + i
                    ps = ps_proj.tile([128, QT], f32, tag="proj", name="pso")
                    nc.tensor.matmul(
                        ps[:, 0:cw],
                        wo_sb[:, ot * 128 : (ot + 1) * 128],
                        xTs[b][:, qs],
                        start=True,
                        stop=True,
                    )
                    nc.vector.tensor_copy(obx[0][:, i, 0:cw], ps[:, 0:cw])
                    if i == 1:
                        dst = outT[
                            op * 256 : (op + 1) * 256,
                            b * N + qt * QT + c0 : b * N + qt * QT + c0 + cw,
                        ].rearrange("(a p) m -> p a m", p=128)
                        nc.sync.dma_start(dst, obx[0][:, :, 0:cw])

                return run

            return stage(0), stage(1)

        class Tctx:
            def __init__(self, b, qt, c0, cw):
                self.b, self.qt, self.c0, self.cw = b, qt, c0, cw
                self.qs = slice(qt * QT + c0, qt * QT + c0 + cw)
                self.pvs = None
                self.pts = [None] * NKT

        def mk_scores(ctx, kt):
            # pair matmul: the two heads run concurrently in PE row groups
            # 0/64; one whole-pair exp on ScalarE (the loop's rate limiter).
            b, cw = ctx.b, ctx.cw
            need(("K", b, kt // 4, kt % 4))
            ks = slice(kt * KT, (kt + 1) * KT)
            sg = ps_s.tile([128, 2, QT], f32, tag="sg", name="sg")
            for h in range(HPC):
                hp = slice(64 * h, 64 * h + 64)
                nc.tensor.matmul(
                    sg[:, h, 0:cw],
                    kTs[b][hp, ks],
                    qTs[b][hp, ctx.qs],
                    start=True,
                    stop=True,
                )
            pt = ppool.tile([128, 2, QT], bf16, tag="p", name="pt")
            nc.scalar.activation(
                pt[:, :, 0:cw],
                sg[:, :, 0:cw],
                mybir.ActivationFunctionType.Exp,
                scale=0.125,
            )
            ctx.pts[kt] = pt

        def mk_pv(ctx, kt, g):
            b, cw = ctx.b, ctx.cw
            if kt == 0:
                ctx.pvs = [
                    ps_pv.tile([128, QT], f32, tag="pv", name=f"pv{h}")
                    for h in range(HPC)
                ]
            need(("V", b, kt))
            pt = ctx.pts[kt]
            ctx.pts[kt] = None
            for h in range(HPC):
                nc.tensor.matmul(
                    ctx.pvs[h][:, 0:cw],
                    vs[b][:, h * NKT + kt, :],
                    pt[:, h, 0:cw],
                    start=(kt == 0),
                    stop=(kt == NKT - 1),
                )
            if kt == NKT - 1:
                finalize(ctx, g)

        def finalize(ctx, g):
            # softmax normalize: values for head h live in psum partitions
            # 64h:64h+64; the replicated denominators (from the ones-blocks
            # in V) at partition 64 (h0) / 32 (h1).  Evacuate psum in two
            # copies right away (frees the PV banks for the next tile); the
            # recip/broadcast/mul + out-projection ride the pending queue.
            # bf16 evacuation: values only need bf16 (output is bf16 anyway)
            # and a bf16 denominator costs ~0.1% RMS -- in exchange the rb
            # broadcast matmuls run at bf16 rate (213ns) instead of fp32's
            # LOW_HIGH double-pass (~1.4us), saving ~21us of PE.
            b, cw, qs = ctx.b, ctx.cw, ctx.qs
            pvs = ctx.pvs
            pvsb0 = npool.tile([128, QT], bf16, tag="pvsb0", name="pvsb0")
            nc.vector.tensor_copy(pvsb0[0:65, 0:cw], pvs[0][0:65, 0:cw])
            pvsb1 = npool.tile([128, QT], bf16, tag="pvsb1", name="pvsb1")
            nc.vector.tensor_copy(pvsb1[64:128, 0:cw], pvs[1][64:128, 0:cw])
            nc.vector.tensor_copy(pvsb1[32:33, 0:cw], pvs[1][32:33, 0:cw])
            rbs = [None, None]

            def rb_mms():
                # broadcast the RAW denominator rows across partitions with
                # 1-contraction PE matmuls into PSUM (works from any base
                # partition, unlike gpsimd partition_broadcast / custom-DVE
                # ops, which silently no-op unless the AP starts at 0)
                for h, (pvsb, dp) in enumerate(
                    ((pvsb0, slice(64, 65)), (pvsb1, slice(32, 33)))
                ):
                    rb = ps_proj.tile([128, QT], f32, tag="proj", name=f"rb{h}")
                    nc.tensor.matmul(
                        rb[64 * h : 64 * h + 64, 0:cw],
                        ones_bc[dp, :],
                        pvsb[dp, 0:cw],
                        start=True,
                        stop=True,
                    )
                    rbs[h] = rb

            def recips():
                # full-height base-0 reciprocals (the custom-DVE op silently
                # no-ops unless the AP starts at partition 0)
                nc.vector.reciprocal_approx_fast(
                    rbs[0][0:64, 0:cw], rbs[0][0:64, 0:cw]
                )
                nc.vector.reciprocal_approx_fast(
                    rbs[1][0:128, 0:cw], rbs[1][0:128, 0:cw]
                )

            def muls():
                nc.vector.tensor_mul(
                    xTs[b][0:64, qs], pvsb0[0:64, 0:cw], rbs[0][0:64, 0:cw]
                )
                nc.vector.tensor_mul(
                    xTs[b][64:128, qs], pvsb1[64:128, 0:cw], rbs[1][64:128, 0:cw]
                )

            pending.append((g + 1, rb_mms, 430))
            pending.append((g + 2, recips, 0))
            pending.append((g + 3, muls, 0))
            for op in range(KC // 2):
                s0, s1 = outproj_stages(b, ctx.qt, op, ctx.c0, cw)
                pending.append((g + 4 + 2 * op, s0, 380 * cw // QT))
                pending.append((g + 5 + 2 * op, s1, 380 * cw // QT))

        # ================= schedule =================
        # warm up b0's first tiles in dependency order, then let the stream
        # pull the rest on demand while pump() spreads filler into the loop.
        # DMA issue order = wire priority: the small weight DMAs and the two
        # chunks on the critical path (xk00 -> K00 -> scores, xq00 -> Q00) go
        # first so scores(0) can start ~16us in instead of ~31us.
        need(("cwk",))
        need(("cwq",))
        need(("cbq",))
        need(("xkm",))
        need(("xq", 0, 0))
        need(("K", 0, 0, 0))
        need(("Q", 0, 0))
        need(("xk", 0, 0))
        need(("K", 0, 0, 1))
        need(("xv", 0, 0))
        need(("cwv",))
        need(("K", 0, 0, 2))
        need(("K", 0, 0, 3))
        need(("xk", 0, 1))
        need(("xq", 0, 1))

        # the final q-tile runs as two column halves so the first half's
        # normalize + out-projection overlap the second half's slots; only a
        # half-width serial tail remains after the last matmul
        tiles = [
            (0, 0, 0, QT),
            (0, 1, 0, QT),
            (0, 2, 0, QT),
            (0, 3, 0, QT),
            (1, 0, 0, QT),
            (1, 1, 0, QT),
            (1, 2, 0, QT),
            (1, 3, 0, QT // 2),
            (1, 3, QT // 2, QT // 4),
            (1, 3, 3 * QT // 4, QT // 4),
        ]
        # V proj for the first key-tiles so tile (0,0)'s pv's don't pull
        # them eagerly into its already-dense slots
        need(("V", 0, 0))
        need(("V", 0, 1))
        need(("V", 0, 2))

        PVLAG = 2
        slots = []
        ctxs = {}
        tile_start = {}
        for b, qt, c0, cw in tiles:
            ctxs[(b, qt, c0)] = Tctx(b, qt, c0, cw)
            tile_start[(b, qt, c0)] = len(slots)
            for kt in range(NKT):
                slots.append((b, qt, c0, kt))

        # ---- deterministic EDF filler schedule -------------------------
        # Deadline = the slot where a unit is first demanded by the stream.
        # Units are packed into slots by remaining PE-slack capacity in
        # deadline order; an overdue unit is forced regardless of capacity
        # (the stream's need() would pull it there anyway).
        tb = {0: 0, 1: min(s for (bb, qq, cc), s in tile_start.items() if bb == 1)}

        def deadline(key):
            nm = key[0]
            if nm == "V":
                return tb[key[1]] + key[2] + PVLAG - 1
            if nm in ("K", "Q"):
                b, xi = key[1], key[2]
                d = tb[b] + (4 * xi if nm == "K" else 16 * xi) - 1
                if nm == "K" and len(key) == 4:
                    d += key[3]  # quarter ch is first needed at kt = 4*xi+ch
                return max(d, 0)
            if nm == "xk":
                return max(tb[key[1]] + 4 * key[2] - 5, 0)
            if nm == "xq":
                return max(tb[key[1]] + 16 * key[2] - 5, 0)
            if nm == "xv":
                return max(tb[key[1]] + 4 * key[2] - 3, 0)
            return 4  # consts (cwo/cwv/...)

        dls = sorted(
            (deadline(k), idx, k)
            for idx, k in enumerate(order)
            if k in units and not (len(k) == 3 and k[0] in ("K", "Q"))
        )
        CAPS = {QT: 400, QT // 2: 280, QT // 4: 200}

        def emit_pv(j, g):
            b, qt, c0, kt = slots[j]
            mk_pv(ctxs[(b, qt, c0)], kt, g)

        ptr = [0]

        def fill(i, cap):
            while ptr[0] < len(dls):
                d, _, key = dls[ptr[0]]
                if key not in units:
                    ptr[0] += 1
                    continue
                cost = units[key][1]
                if d <= i or cost <= cap:
                    cap -= cost
                    need(key)
                    ptr[0] += 1
                else:
                    break

        for i, (b, qt, c0, kt) in enumerate(slots):
            if kt == 0:
                cw_t = ctxs[(b, qt, c0)].cw
                for ch in range(c0 // 128, (c0 + cw_t) // 128):
                    need(("Q", b, qt, ch))
            mk_scores(ctxs[(b, qt, c0)], kt)
            if i >= PVLAG:
                emit_pv(i - PVLAG, i)
            staged_cost = 0
            while pending and pending[0][0] <= i:
                _, fn, cost = pending.pop(0)
                fn()
                staged_cost += cost
            fill(i, CAPS[ctxs[(b, qt, c0)].cw] - staged_cost)
        for j in range(len(slots) - PVLAG, len(slots)):
            emit_pv(j, len(slots))
        drain()
        while pending:
            pending.pop(0)[1]()

    nc.compile()
    _cache["nc"] = nc
    return nc


def kernel(x_q, x_k, x_v, Wq, bq, Wk, bk, Wv, bv, Wo, bo, _trace=False):
    x_q = np.asarray(x_q, dtype=np.float32)
    x_k = np.asarray(x_k, dtype=np.float32)
    x_v = np.asarray(x_v, dtype=np.float32)
    Wq, Wk, Wv, Wo = (np.asarray(w, dtype=np.float32) for w in (Wq, Wk, Wv, Wo))
    bq, bk, bv, bo = (np.asarray(v, dtype=np.float32) for v in (bq, bk, bv, bo))

    bf = ml_dtypes.bfloat16
    xqT = np.ascontiguousarray(x_q.reshape(R, D).T).astype(bf)
    xkT = np.ascontiguousarray(x_k.reshape(R, D).T).astype(bf)
    xvT = np.ascontiguousarray(x_v.reshape(R, D).T).astype(bf)

    in_maps = []
    for c in range(NC):
        s = slice(DHC * c, DHC * (c + 1))
        in_maps.append(
            {
                "xqT": xqT,
                "xkT": xkT,
                "xvT": xvT,
                "wqT": _foldw(Wq[s, :].T).astype(bf),
                "wkT": _foldw(Wk[s, :].T).astype(bf),
                "wvT": _foldw(Wv[s, :].T).astype(bf),
                "woT": np.ascontiguousarray(Wo[:, s].T).astype(bf),
                "bq": bq[s][:, None].copy(),
            }
        )

    nc = build()
    res = run_bass_kernel_spmd(nc, in_maps, core_ids=list(range(NC)), trace=_trace)
    total = np.zeros((D, R), dtype=np.float32)
    for c in range(NC):
        total += res.results[c]["outT"].astype(np.float32)
    # bv is not applied on device: softmax weights sum to 1, so the V bias
    # contributes exactly bv @ Wo.T to every output row -- fold it into bo.
    out = total.T + (bo + bv @ Wo.T)[None, :]
    if _trace:
        kernel.last_exec_time_ns = res.exec_time_ns
    return out.reshape(B, N, D).astype(np.float32)

